# revision 33
# baseline (speedup 1.0000x reference)
"""Trainium2 Bass kernel for nn_Attention_59528246723073.

Reference (per batch b, channel c; x[b,c] is [S=256, T=64]):
    fs = tanh(x @ Wspect[c])            # [S]   (contract T)
    ft = tanh(x.T @ Wtemp[c])           # [T]   (contract S)
    a  = softmax_S(fs) * 100
    g  = softmax_T(ft)
    out[b,c,s,t] = x[b,c,s,t] * a[s] * g[t]

Distribution: data-parallel over batch B=32 -> 4 per core on 8 cores.

Per-core layout: for each local batch b, SBUF tile [128 part = channels,
S*T free] (x[b] is exactly this, contiguous), loaded with an SWDGE
cast-DMA (f32 in HBM -> fp16 in SBUF).  All big elementwise ops run on
VectorE in fp16 with the 2x_1p perf mode (innermost step 1 on every
operand):
  - fs-mul multiplies by Wspect broadcast over s (inner t contiguous),
  - ft-mul multiplies by a pre-materialized Wtemp replica (contiguous, flat),
  - ft reduction = flat in-place fold chain over s, accumulated across chunks,
  - fs reduction = in-place folds over t (to width 4) + one f32 tensor_reduce,
  - final: g-mul (inner-contiguous bcast) then a-mul via a paired-duplicate
    a2[p, 2s+j] = a[p,s] so the broadcast keeps innermost step 1.
Softmax skips the max-subtraction: logits are tanh outputs in [-1, 1], so
exp never overflows and the exp's fused accum_out provides the sum.
Output stays fp16 in SBUF and is cast to f32 by the SWDGE output DMA.
The first batch uses graduated chunk sizes (32,32,64,128 s-rows) so compute
starts as soon as 1 MB has landed; the last batch's finals are split into
eighths so the tail out-DMA exposure is short.
"""

import numpy as np

import concourse.bass as bass
import concourse.tile as tile
from concourse import bacc, mybir
from concourse.bass_utils import run_bass_kernel_spmd

B, C, S, T = 32, 128, 256, 64
N_CORES = 8
B_LOC = B // N_CORES
F32 = mybir.dt.float32
F16 = mybir.dt.float16

_NC = None


def build_nc():
    nc = bacc.Bacc("TRN2", target_bir_lowering=False, debug=False)
    x = nc.dram_tensor("x", [B_LOC, C, S, T], F32, kind="ExternalInput")
    ws = nc.dram_tensor("wspect", [C, T], F32, kind="ExternalInput")
    wt = nc.dram_tensor("wtemp", [C, S], F32, kind="ExternalInput")
    out = nc.dram_tensor("out", [B_LOC, C, S, T], F32, kind="ExternalOutput")

    AF = mybir.ActivationFunctionType
    OP = mybir.AluOpType
    AX = mybir.AxisListType

    with tile.TileContext(nc) as tc:
        with (
            tc.tile_pool(name="consts", bufs=1) as cpool,
            tc.tile_pool(name="x2", bufs=2) as x2pool,
            tc.tile_pool(name="tmp", bufs=3) as tpool,
            tc.tile_pool(name="tmp2", bufs=4) as tpool2,
            tc.tile_pool(name="ocp", bufs=3) as ocpool,
            tc.tile_pool(name="small", bufs=2) as spool,
        ):
            # --- constants: weights in fp16; Wtemp replicated along t ---
            # weights straight to fp16 via SWDGE cast-DMA (no staging hop)
            ws16 = cpool.tile([C, T], F16)
            nc.gpsimd.dma_start(ws16[:], ws[:])
            wt16 = cpool.tile([C, S], F16)
            nc.gpsimd.dma_start(wt16[:], wt[:])
            # wt_rep[c, s, t] = Wtemp[c, s] (fp16 contiguous), built in four
            # pieces on ScalarE so the first ft-mul isn't gated on one build.
            wt_rep = cpool.tile([C, S * T], F16)
            wt_rep3 = wt_rep.rearrange("p (s t) -> p s t", t=T)
            for q in range(4):
                sq = slice(q * S // 4, (q + 1) * S // 4)
                nc.scalar.activation(
                    wt_rep3[:, sq, :],
                    wt16[:, sq].unsqueeze(2).to_broadcast((C, S // 4, T)),
                    AF.Copy,
                )

            for b in range(B_LOC):
                X2 = x2pool.tile([C, S * T], F16, tag="X2")
                X23 = X2.rearrange("p (s t) -> p s t", t=T)
                fs = spool.tile([C, S], F32, tag="fs")
                ft = spool.tile([C, T], F32, tag="ft")

                # graduated chunks on the first batch: compute starts after
                # the first 1 MB lands instead of after 4 MB.
                chunks = (32, 32, 64, 128) if b == 0 else (128, 128)
                s0 = 0
                for k, sc in enumerate(chunks):
                    sl = slice(s0, s0 + sc)
                    fsl = slice(s0 * T, (s0 + sc) * T)
                    # load with SWDGE cast f32 -> fp16, <=1MB pieces
                    with nc.named_scope("load"):
                        for q0 in range(s0, s0 + sc, 32):
                            sq = slice(q0, min(q0 + 32, s0 + sc))
                            nc.gpsimd.dma_start(
                                X2[:, sq.start * T : sq.stop * T],
                                x[b, :, sq, :],
                            )

                    xc = X23[:, sl, :]
                    xcf = X2[:, fsl]
                    # ft += sum_{s in chunk} xc * Wtemp[:, sl, None]
                    # Split the product into two half-tiles; the first fold
                    # (half + half) runs on the DMA engines' CCE adders, the
                    # rest is a flat fp16 fold chain on VectorE.
                    with nc.named_scope("ft"):
                        h = sc * T // 2
                        tmp2 = tpool2.tile([C, h], F16, tag="tmph")
                        tmp2b = tpool2.tile([C, h], F16, tag="tmph")
                        nc.vector.tensor_tensor(
                            tmp2[:], X2[:, fsl.start : fsl.start + h],
                            wt_rep[:, fsl.start : fsl.start + h], op=OP.mult,
                        )
                        nc.vector.tensor_tensor(
                            tmp2b[:], X2[:, fsl.start + h : fsl.stop],
                            wt_rep[:, fsl.start + h : fsl.stop], op=OP.mult,
                        )
                        nc.vector.tensor_tensor(tmp2[:], tmp2[:], tmp2b[:], op=OP.add)
                        w = h // 2
                        while w >= T:
                            nc.vector.tensor_tensor(
                                tmp2[:, 0:w], tmp2[:, 0:w], tmp2[:, w : 2 * w],
                                op=OP.add,
                            )
                            w //= 2
                        if k == 0:
                            nc.vector.tensor_copy(ft[:], tmp2[:, 0:T])
                        else:
                            nc.vector.tensor_tensor(
                                ft[:], ft[:], tmp2[:, 0:T], op=OP.add
                            )
                    # fs[:, sl] = sum_t xc * Wspect[:, None, :]
                    with nc.named_scope("fs"):
                        tmp = tpool.tile([C, sc * T], F16, tag="tmp")
                        t3 = tmp.rearrange("p (s t) -> p s t", t=T)
                        nc.vector.tensor_tensor(
                            t3, xc, ws16.unsqueeze(1).to_broadcast((C, sc, T)),
                            op=OP.mult,
                        )
                        w = T // 2
                        while w >= 4:
                            nc.vector.tensor_tensor(
                                t3[:, :, 0:w], t3[:, :, 0:w],
                                t3[:, :, w : 2 * w], op=OP.add,
                            )
                            w //= 2
                        nc.vector.reduce_sum(fs[:, sl], t3[:, :, 0:4], axis=AX.X)
                    s0 += sc

                with nc.named_scope("softmax"):
                    # logits are tanh outputs in [-1,1]: no max-subtraction
                    # needed; exp's fused accum_out gives the softmax sum.
                    ssum = spool.tile([C, 1], F32, tag="ssum")
                    rec = spool.tile([C, 1], F32, tag="rec")
                    nc.scalar.activation(fs[:], fs[:], AF.Tanh)
                    nc.scalar.activation(
                        fs[:], fs[:], AF.Exp, accum_out=ssum[:, 0:1]
                    )
                    nc.vector.reciprocal(rec[:], ssum[:])
                    a2 = spool.tile([C, 2 * S], F16, tag="a2")
                    nc.vector.tensor_scalar(
                        out=a2.rearrange("p (s j) -> p s j", j=2),
                        in0=fs[:].unsqueeze(2).to_broadcast((C, S, 2)),
                        scalar1=rec[:, 0:1], scalar2=100.0,
                        op0=OP.mult, op1=OP.mult,
                    )

                    ssum2 = spool.tile([C, 1], F32, tag="ssum2")
                    rec2 = spool.tile([C, 1], F32, tag="rec2")
                    nc.scalar.activation(ft[:], ft[:], AF.Tanh)
                    nc.scalar.activation(
                        ft[:], ft[:], AF.Exp, accum_out=ssum2[:, 0:1]
                    )
                    nc.vector.reciprocal(rec2[:], ssum2[:])
                    g16 = spool.tile([C, T], F16, tag="g16")
                    nc.vector.tensor_scalar(
                        out=g16[:], in0=ft[:], scalar1=rec2[:, 0:1],
                        scalar2=None, op0=OP.mult,
                    )

                # final multiplies + store; eighths on the last batch so the
                # tail out-DMA exposure is short.
                nf = 8 if b == B_LOC - 1 else 4
                SQ = S // nf
                g_bcq = g16.unsqueeze(1).to_broadcast((C, SQ, T))
                for k in range(nf):
                    sl = slice(k * SQ, (k + 1) * SQ)
                    with nc.named_scope("final"):
                        oc = ocpool.tile([C, SQ * T], F16, tag="oc")
                        o3 = oc.rearrange("p (s t) -> p s t", t=T)
                        # a-mul first (reads X2, fp16 pairs keep 2x), then
                        # g-mul in place
                        oP = oc.rearrange(
                            "p (s pr j) -> p s pr j", pr=T // 2, j=2
                        )
                        xP = X2[
                            :, sl.start * T : sl.stop * T
                        ].rearrange("p (s pr j) -> p s pr j", pr=T // 2, j=2)
                        aP = (
                            a2[:, 2 * k * SQ : 2 * (k + 1) * SQ]
                            .rearrange("p (s j) -> p s j", j=2)
                            .unsqueeze(2)
                            .to_broadcast((C, SQ, T // 2, 2))
                        )
                        nc.vector.tensor_tensor(oP, xP, aP, op=OP.mult)
                        nc.vector.tensor_tensor(o3, o3, g_bcq, op=OP.mult)
                        # SWDGE cast fp16 -> f32 on the way out
                        nc.gpsimd.dma_start(out[b, :, sl, :], oc[:])

    nc.compile()
    return nc


def get_nc():
    global _NC
    if _NC is None:
        _NC = build_nc()
    return _NC


def shard_inputs(x, Wspect, Wtemp):
    ws = np.ascontiguousarray(Wspect.reshape(C, T).astype(np.float32))
    wt = np.ascontiguousarray(Wtemp.reshape(C, S).astype(np.float32))
    x = np.ascontiguousarray(x.astype(np.float32))
    return [
        {"x": x[i * B_LOC : (i + 1) * B_LOC], "wspect": ws, "wtemp": wt}
        for i in range(N_CORES)
    ]


def unshard(results):
    return np.concatenate([r["out"] for r in results], axis=0)


def kernel(x, Wspect, Wtemp):
    nc = get_nc()
    in_maps = shard_inputs(x, Wspect, Wtemp)
    res = run_bass_kernel_spmd(nc, in_maps, core_ids=list(range(N_CORES)))
    return unshard(res.results)


# revision 34
# speedup vs baseline: 1.0137x; 1.0137x over previous
"""Trainium2 Bass kernel for nn_Attention_59528246723073.

Reference (per batch b, channel c; x[b,c] is [S=256, T=64]):
    fs = tanh(x @ Wspect[c])            # [S]   (contract T)
    ft = tanh(x.T @ Wtemp[c])           # [T]   (contract S)
    a  = softmax_S(fs) * 100
    g  = softmax_T(ft)
    out[b,c,s,t] = x[b,c,s,t] * a[s] * g[t]

Distribution: data-parallel over batch B=32 -> 4 per core on 8 cores.

Per-core layout: for each local batch b, SBUF tile [128 part = channels,
S*T free] (x[b] is exactly this, contiguous), loaded with an SWDGE
cast-DMA (f32 in HBM -> fp16 in SBUF).  All big elementwise ops run on
VectorE in fp16 with the 2x_1p perf mode (innermost step 1 on every
operand):
  - fs-mul multiplies by Wspect broadcast over s (inner t contiguous),
  - ft-mul multiplies by a pre-materialized Wtemp replica (contiguous, flat),
  - ft reduction = flat in-place fold chain over s, accumulated across chunks,
  - fs reduction = in-place folds over t (to width 4) + one f32 tensor_reduce,
  - final: g-mul (inner-contiguous bcast) then a-mul via a paired-duplicate
    a2[p, 2s+j] = a[p,s] so the broadcast keeps innermost step 1.
Softmax skips the max-subtraction: logits are tanh outputs in [-1, 1], so
exp never overflows and the exp's fused accum_out provides the sum.
Output stays fp16 in SBUF and is cast to f32 by the SWDGE output DMA.
The first batch uses graduated chunk sizes (32,32,64,128 s-rows) so compute
starts as soon as 1 MB has landed; the last batch's finals are split into
eighths so the tail out-DMA exposure is short.
"""

import numpy as np

import concourse.bass as bass
import concourse.tile as tile
from concourse import bacc, mybir
from concourse.bass_utils import run_bass_kernel_spmd

B, C, S, T = 32, 128, 256, 64
N_CORES = 8
B_LOC = B // N_CORES
F32 = mybir.dt.float32
F16 = mybir.dt.float16

_NC = None


def build_nc():
    nc = bacc.Bacc("TRN2", target_bir_lowering=False, debug=False)
    x = nc.dram_tensor("x", [B_LOC, C, S, T], F32, kind="ExternalInput")
    ws = nc.dram_tensor("wspect", [C, T], F32, kind="ExternalInput")
    wt = nc.dram_tensor("wtemp", [C, S], F32, kind="ExternalInput")
    out = nc.dram_tensor("out", [B_LOC, C, S, T], F32, kind="ExternalOutput")

    AF = mybir.ActivationFunctionType
    OP = mybir.AluOpType
    AX = mybir.AxisListType

    with tile.TileContext(nc) as tc:
        with (
            tc.tile_pool(name="consts", bufs=1) as cpool,
            tc.tile_pool(name="x2", bufs=2) as x2pool,
            tc.tile_pool(name="tmp", bufs=3) as tpool,
            tc.tile_pool(name="tmp2", bufs=4) as tpool2,
            tc.tile_pool(name="ocp", bufs=3) as ocpool,
            tc.tile_pool(name="small", bufs=2) as spool,
        ):
            # --- constants: weights in fp16; Wtemp replicated along t ---
            # weights straight to fp16 via SWDGE cast-DMA (no staging hop)
            ws16 = cpool.tile([C, T], F16)
            nc.gpsimd.dma_start(ws16[:], ws[:])
            wt16 = cpool.tile([C, S], F16)
            nc.gpsimd.dma_start(wt16[:], wt[:])
            # wt_rep[c, s, t] = Wtemp[c, s] (fp16 contiguous), built in four
            # pieces on ScalarE so the first ft-mul isn't gated on one build.
            wt_rep = cpool.tile([C, S * T], F16)
            wt_rep3 = wt_rep.rearrange("p (s t) -> p s t", t=T)
            for q in range(4):
                sq = slice(q * S // 4, (q + 1) * S // 4)
                nc.scalar.activation(
                    wt_rep3[:, sq, :],
                    wt16[:, sq].unsqueeze(2).to_broadcast((C, S // 4, T)),
                    AF.Copy,
                )

            for b in range(B_LOC):
                X2 = x2pool.tile([C, S * T], F16, tag="X2")
                X23 = X2.rearrange("p (s t) -> p s t", t=T)
                fs = spool.tile([C, S], F32, tag="fs")
                ft = spool.tile([C, T], F32, tag="ft")

                # graduated chunks on the first batch: compute starts after
                # the first 1 MB lands instead of after 4 MB.
                chunks = (32, 32, 64, 128) if b == 0 else (128, 128)
                s0 = 0
                for k, sc in enumerate(chunks):
                    sl = slice(s0, s0 + sc)
                    fsl = slice(s0 * T, (s0 + sc) * T)
                    # load with SWDGE cast f32 -> fp16, <=1MB pieces
                    with nc.named_scope("load"):
                        for q0 in range(s0, s0 + sc, 32):
                            sq = slice(q0, min(q0 + 32, s0 + sc))
                            nc.gpsimd.dma_start(
                                X2[:, sq.start * T : sq.stop * T],
                                x[b, :, sq, :],
                            )

                    xc = X23[:, sl, :]
                    xcf = X2[:, fsl]
                    # ft += sum_{s in chunk} xc * Wtemp[:, sl, None]
                    # Split the product into two half-tiles; the first fold
                    # (half + half) runs on the DMA engines' CCE adders, the
                    # rest is a flat fp16 fold chain on VectorE.
                    with nc.named_scope("ft"):
                        h = sc * T // 2
                        tmp2 = tpool2.tile([C, h], F16, tag="tmph")
                        tmp2b = tpool2.tile([C, h], F16, tag="tmph")
                        nc.vector.tensor_tensor(
                            tmp2[:], X2[:, fsl.start : fsl.start + h],
                            wt_rep[:, fsl.start : fsl.start + h], op=OP.mult,
                        )
                        nc.vector.tensor_tensor(
                            tmp2b[:], X2[:, fsl.start + h : fsl.stop],
                            wt_rep[:, fsl.start + h : fsl.stop], op=OP.mult,
                        )
                        nc.vector.tensor_tensor(tmp2[:], tmp2[:], tmp2b[:], op=OP.add)
                        w = h // 2
                        while w >= T:
                            nc.vector.tensor_tensor(
                                tmp2[:, 0:w], tmp2[:, 0:w], tmp2[:, w : 2 * w],
                                op=OP.add,
                            )
                            w //= 2
                        if k == 0:
                            nc.vector.tensor_copy(ft[:], tmp2[:, 0:T])
                        else:
                            nc.vector.tensor_tensor(
                                ft[:], ft[:], tmp2[:, 0:T], op=OP.add
                            )
                    # fs[:, sl] = sum_t xc * Wspect[:, None, :]
                    with nc.named_scope("fs"):
                        tmp = tpool.tile([C, sc * T], F16, tag="tmp")
                        t3 = tmp.rearrange("p (s t) -> p s t", t=T)
                        nc.vector.tensor_tensor(
                            t3, xc, ws16.unsqueeze(1).to_broadcast((C, sc, T)),
                            op=OP.mult,
                        )
                        w = T // 2
                        while w >= 2:
                            nc.vector.tensor_tensor(
                                t3[:, :, 0:w], t3[:, :, 0:w],
                                t3[:, :, w : 2 * w], op=OP.add,
                            )
                            w //= 2
                        nc.vector.reduce_sum(fs[:, sl], t3[:, :, 0:2], axis=AX.X)
                    s0 += sc

                with nc.named_scope("softmax"):
                    # logits are tanh outputs in [-1,1]: no max-subtraction
                    # needed; exp's fused accum_out gives the softmax sum.
                    ssum = spool.tile([C, 1], F32, tag="ssum")
                    rec = spool.tile([C, 1], F32, tag="rec")
                    nc.scalar.activation(fs[:], fs[:], AF.Tanh)
                    nc.scalar.activation(
                        fs[:], fs[:], AF.Exp, accum_out=ssum[:, 0:1]
                    )
                    nc.vector.reciprocal(rec[:], ssum[:])
                    a2 = spool.tile([C, 2 * S], F16, tag="a2")
                    nc.vector.tensor_scalar(
                        out=a2.rearrange("p (s j) -> p s j", j=2),
                        in0=fs[:].unsqueeze(2).to_broadcast((C, S, 2)),
                        scalar1=rec[:, 0:1], scalar2=100.0,
                        op0=OP.mult, op1=OP.mult,
                    )

                    ssum2 = spool.tile([C, 1], F32, tag="ssum2")
                    rec2 = spool.tile([C, 1], F32, tag="rec2")
                    nc.scalar.activation(ft[:], ft[:], AF.Tanh)
                    nc.scalar.activation(
                        ft[:], ft[:], AF.Exp, accum_out=ssum2[:, 0:1]
                    )
                    nc.vector.reciprocal(rec2[:], ssum2[:])
                    g16 = spool.tile([C, T], F16, tag="g16")
                    nc.vector.tensor_scalar(
                        out=g16[:], in0=ft[:], scalar1=rec2[:, 0:1],
                        scalar2=None, op0=OP.mult,
                    )

                # final multiplies + store; eighths on the last batch so the
                # tail out-DMA exposure is short.
                nf = 8 if b == B_LOC - 1 else 4
                SQ = S // nf
                g_bcq = g16.unsqueeze(1).to_broadcast((C, SQ, T))
                for k in range(nf):
                    sl = slice(k * SQ, (k + 1) * SQ)
                    with nc.named_scope("final"):
                        oc = ocpool.tile([C, SQ * T], F16, tag="oc")
                        o3 = oc.rearrange("p (s t) -> p s t", t=T)
                        nc.vector.tensor_tensor(
                            o3, X23[:, sl, :], g_bcq, op=OP.mult
                        )
                        # a-mul on fp16 pairs: innermost step-1 j keeps 2x
                        oP = oc.rearrange(
                            "p (s pr j) -> p s pr j", pr=T // 2, j=2
                        )
                        aP = (
                            a2[:, 2 * k * SQ : 2 * (k + 1) * SQ]
                            .rearrange("p (s j) -> p s j", j=2)
                            .unsqueeze(2)
                            .to_broadcast((C, SQ, T // 2, 2))
                        )
                        nc.vector.tensor_tensor(oP, oP, aP, op=OP.mult)
                        # SWDGE cast fp16 -> f32 on the way out
                        nc.gpsimd.dma_start(out[b, :, sl, :], oc[:])

    nc.compile()
    return nc


def get_nc():
    global _NC
    if _NC is None:
        _NC = build_nc()
    return _NC


def shard_inputs(x, Wspect, Wtemp):
    ws = np.ascontiguousarray(Wspect.reshape(C, T).astype(np.float32))
    wt = np.ascontiguousarray(Wtemp.reshape(C, S).astype(np.float32))
    x = np.ascontiguousarray(x.astype(np.float32))
    return [
        {"x": x[i * B_LOC : (i + 1) * B_LOC], "wspect": ws, "wtemp": wt}
        for i in range(N_CORES)
    ]


def unshard(results):
    return np.concatenate([r["out"] for r in results], axis=0)


def kernel(x, Wspect, Wtemp):
    nc = get_nc()
    in_maps = shard_inputs(x, Wspect, Wtemp)
    res = run_bass_kernel_spmd(nc, in_maps, core_ids=list(range(N_CORES)))
    return unshard(res.results)
